# revision 11
# baseline (speedup 1.0000x reference)
"""CAMSA multi-mask attention kernel for one TRN2 chip (8 NeuronCores).

Problem: B=4, S=2048, D=1024, M=4 stride masks.
  Q = x@Wq + bq ; K = x@Wk + bk ; V = x@Wv + bv     (biases are zero-fill)
  scores = Q K^T / sqrt(D)                           [B,S,S]
  weights_m = softmax(where(mask_m==0, -1e9, scores))
  out = (mean_m weights_m) @ V @ Wo + bo

Algebra: with P = exp(scores/sqrt(D)) (no row-max needed; scores ~ N(0,1)):
  den_m[q] = sum_k mask_m[q,k] P[q,k];  inv_m = 1/den_m
  Wsum = sum_m inv_m * (mask_m*P);  out = Wsum @ V @ (Wo/M)
(the 1/M mean is folded into Wo on the host).

Sharding: core c = (batch b=c//2, query-half h=c%2): 1024 query rows,
full 2048 keys; K/V projections duplicated within a batch pair (no
collectives).  Host-side prep is pure dtype/layout: bf16 casts of
x/W (the device would DMA-cast anyway), masks int32 -> uint8 in a
per-q-tile layout, and a per-core "own half first" permutation of the
key axis (applied consistently to x columns and mask k) so one SPMD
program serves both halves without a separate xTq input.

Device pipeline per core (all matmuls bf16, contraction on partitions):
  tensor order: Q proj, K proj, V proj, scores t=0..7, AV 2-tile
  groups interleaved with the Wo projection per q-tile.
  per q-tile softmax chain under the matmul stream:
    ACT: P=exp(scores) from PSUM (4 blocks), 2 scaled copies
    DVE: 3x STT(mask*P, accum->den), recip, 2x TS(4x), 3x TT(2x)
    GpSimd: 1x STT(mask*P, accum->den)
    sync-DMA: Wsum -> WT transpose (xbar), outputs
"""

import numpy as np

B, S, D, M = 4, 2048, 1024, 4
SQ = S // 2          # query rows per core
PART = 128
N_CORES = 8

_CACHE = {}


def build(nc_factory=None, S=S, D=D, SQ=SQ, M=M, use_deps=True):
    from concourse import bass, mybir, bacc, tile
    from concourse.tile import add_dep_helper

    fp32 = mybir.dt.float32
    bf16 = mybir.dt.bfloat16
    u8 = mybir.dt.uint8
    AF = mybir.ActivationFunctionType
    ALU = mybir.AluOpType

    P = PART
    DCH = D // P         # d-chunks (8)
    KCH = S // P         # key-row chunks (16)
    QTILES = SQ // P     # q-tiles per core (8)
    NB = 512

    if nc_factory is None:
        nc = bacc.Bacc("TRN2", target_bir_lowering=False, debug=False,
                       num_devices=N_CORES)
    else:
        nc = nc_factory()

    xT_d = nc.dram_tensor("xT", [D, S], bf16, kind="ExternalInput")
    mk_d = nc.dram_tensor("mk", [QTILES, P, M * S], u8, kind="ExternalInput")
    wq_d = nc.dram_tensor("Wq", [D, D], bf16, kind="ExternalInput")
    wk_d = nc.dram_tensor("Wk", [D, D], bf16, kind="ExternalInput")
    wv_d = nc.dram_tensor("Wv", [D, D], bf16, kind="ExternalInput")
    wo_d = nc.dram_tensor("Wo", [D, D], bf16, kind="ExternalInput")
    out_d = nc.dram_tensor("out", [SQ, D], fp32, kind="ExternalOutput")

    with tile.TileContext(nc) as tc:
        with tc.tile_pool(name="persist", bufs=1) as pp, \
             tc.tile_pool(name="psum", bufs=6, space="PSUM") as psp, \
             tc.tile_pool(name="psav", bufs=2, space="PSUM") as psav:

            QT = pp.tile([P, DCH * SQ], bf16, tag="QT")  # [p, j*SQ+q] = Q[q, j*128+p]
            KT = pp.tile([P, DCH * S], bf16, tag="KT")  # [p,j*S+k] = K[k,j*128+p]
            V = pp.tile([P, KCH * D], bf16)      # [p, i*D+d]  = V[i*128+p, d]
            WT = pp.tile([P, KCH * SQ], bf16, name="WT", tag="WT")
            #    [p, i*SQ+q] = Wsum[q, i*128+p]

            def wload(dst, src_d):
                return nc.gpsimd.dma_start(
                    dst[:].rearrange("p (c d) -> p c d", c=DCH),
                    src_d.ap().rearrange("(c p) d -> p c d", p=P))

            # PSUM -> SBUF copy engines, round-robined so no engine stalls
            # the tensor stream (GPSIMD cannot access PSUM).
            cp_engines = [nc.vector, nc.scalar]
            cp_idx = [0]

            def psum_copy(dst_ap, src_ap):
                eng = cp_engines[cp_idx[0] % 2]
                cp_idx[0] += 1
                if eng is nc.scalar:
                    eng.copy(dst_ap, src_ap)
                else:
                    eng.tensor_copy(dst_ap, src_ap)

            def proj(dst, w_sb, src_sb, ncols, src_off=0):
                # dst[p, j*ncols+r] = sum_dx W[dx, j*128+p] * src[dx, src_off+r]
                for j in range(DCH):
                    for qb in range(ncols // NB):
                        ps = psp.tile([P, NB], fp32, tag="ps", name="ps")
                        for c in range(DCH):
                            nc.tensor.matmul(
                                ps[:],
                                w_sb[:, c * D + j * P: c * D + (j + 1) * P],
                                src_sb[:, c * S + src_off + qb * NB:
                                       c * S + src_off + (qb + 1) * NB],
                                start=(c == 0), stop=(c == DCH - 1))
                        psum_copy(
                            dst[:, j * ncols + qb * NB: j * ncols + (qb + 1) * NB],
                            ps[:])

            # ---- load + Q/K projections --------------------------------
            sx_ctx = tc.tile_pool(name="stage_x", bufs=1)
            sx = sx_ctx.__enter__()
            XT = sx.tile([P, DCH * S], bf16, name="XT")
            wqk_ctx = tc.tile_pool(name="stage_wqk", bufs=1)
            swqk = wqk_ctx.__enter__()
            Wq = swqk.tile([P, DCH * D], bf16, name="Wq")
            Wk = swqk.tile([P, DCH * D], bf16, name="Wk")
            d_wq = wload(Wq, wq_d)
            # own-half query columns first (host permutation): split the XT
            # load so the Q projection can start after ~4MB of DMA.
            d_xh = nc.gpsimd.dma_start(
                XT[:].rearrange("p (c r) -> p c r", c=DCH)[:, :, 0:SQ],
                xT_d.ap().rearrange("(c p) r -> p c r", p=P)[:, :, 0:SQ])
            d_xt = nc.gpsimd.dma_start(
                XT[:].rearrange("p (c r) -> p c r", c=DCH)[:, :, SQ:S],
                xT_d.ap().rearrange("(c p) r -> p c r", p=P)[:, :, SQ:S])
            d_wk = wload(Wk, wk_d)
            if use_deps:
                add_dep_helper(d_xh.ins, d_wq.ins, sync=False, reason="dma order")
                add_dep_helper(d_xt.ins, d_xh.ins, sync=False, reason="dma order")
                add_dep_helper(d_wk.ins, d_xt.ins, sync=False, reason="dma order")

            # warm the exp activation table during the projection phase
            with tc.tile_pool(name="warm", bufs=1) as wpool:
                wt = wpool.tile([P, 2], fp32)
                nc.vector.memset(wt[:], 0.0)
                nc.scalar.activation(wt[:], wt[:], AF.Exp)

            proj(QT, Wq, XT, SQ, src_off=0)
            proj(KT, Wk, XT, S, src_off=0)
            wqk_ctx.__exit__(None, None, None)

            # Wv reuses the Wq/Wk space (pool opened after wqk closes); its
            # load waits for the last Wq/Wk reader automatically.
            wv_ctx = tc.tile_pool(name="stage_wv", bufs=1)
            swv = wv_ctx.__enter__()
            Wv = swv.tile([P, DCH * D], bf16, name="Wv")
            d_wv = wload(Wv, wv_d)
            if use_deps:
                add_dep_helper(d_wv.ins, d_wk.ins, sync=False, reason="dma order")
            d_prev = d_wv

            # ---- work pools for the softmax/AV/out phases ----------------
            wk_ctx = tc.tile_pool(name="work", bufs=2)
            wkp = wk_ctx.__enter__()

            # ---- scores -> P -> masked softmax -> WsumT ------------------
            inv_scale = 1.0 / float(np.sqrt(np.float32(D)))
            for t in range(QTILES):
                mt = wkp.tile([P, M * S], bf16, tag="mt", name="mt", bufs=1)
                d_mt = nc.gpsimd.dma_start(mt[:], mk_d.ap()[t])
                if use_deps:
                    add_dep_helper(d_mt.ins, d_prev.ins, sync=False,
                                   reason="mask order")
                    d_prev = d_mt

                Pt = wkp.tile([P, S], bf16, tag="Pt", name="Pt")
                for kb in range(S // NB):
                    ps = psp.tile([P, NB], fp32, tag="ps", name="ps")
                    for c in range(DCH):
                        nc.tensor.matmul(
                            ps[:],
                            QT[:, c * SQ + t * P: c * SQ + (t + 1) * P],
                            KT[:, c * S + kb * NB: c * S + (kb + 1) * NB],
                            start=(c == 0), stop=(c == DCH - 1))
                    nc.scalar.activation(
                        Pt[:, kb * NB:(kb + 1) * NB], ps[:],
                        AF.Exp, scale=inv_scale)

                den = wkp.tile([P, M], fp32, tag="den", name="den")
                # fused product + row-sum per mask; in-place T_m = mask_m*P.
                # (STT only exists on DVE; GPSIMD takes the final add below.)
                for m in range(M):
                    eng = nc.vector
                    eng.scalar_tensor_tensor(
                        out=mt[:, m * S:(m + 1) * S],
                        in0=mt[:, m * S:(m + 1) * S],
                        scalar=1.0, in1=Pt[:],
                        op0=ALU.mult, op1=ALU.mult,
                        accum_out=den[:, m:m + 1])
                inv = wkp.tile([P, M], fp32, tag="inv", name="inv")
                nc.vector.reciprocal(inv[:], den[:])

                # Wsum = sum_m inv_m * T_m as a balanced tree:
                #   A = inv0*T0 (ACT)   Bc = inv1*T1 (DVE TS 4x)
                #   C = inv2*T2 (ACT)   Dc = inv3*T3 (DVE TS 4x)
                #   A += Bc ; C += Dc ; A += C   (DVE TT 2x)
                A = wkp.tile([P, S], bf16, tag="A", name="A")
                Bc = wkp.tile([P, S], bf16, tag="Bc", name="Bc", bufs=1)
                C = wkp.tile([P, S], bf16, tag="C", name="C", bufs=1)
                Dc = wkp.tile([P, S], bf16, tag="Bc", name="Bc", bufs=1)
                nc.scalar.activation(A[:], mt[:, 0:S], AF.Copy,
                                     scale=inv[:, 0:1])
                nc.vector.tensor_scalar(Bc[:], mt[:, S:2 * S],
                                        inv[:, 1:2], None, ALU.mult)
                nc.scalar.activation(C[:], mt[:, 2 * S:3 * S], AF.Copy,
                                     scale=inv[:, 2:3])
                nc.vector.tensor_scalar(Dc[:], mt[:, 3 * S:4 * S],
                                        inv[:, 3:4], None, ALU.mult)
                nc.vector.tensor_tensor(A[:], A[:], Bc[:], op=ALU.add)
                nc.vector.tensor_tensor(C[:], C[:], Dc[:], op=ALU.add)
                nc.gpsimd.tensor_tensor(A[:], A[:], C[:], op=ALU.add)

                # transpose Wsum [128, S] -> WT columns via xbar DMA
                nc.sync.dma_start_transpose(
                    WT[:].rearrange("p (i q) -> p i q", i=KCH)[:, :, t * P:(t + 1) * P],
                    A[:])

            # ---- V projection (after scores: DVE chain overlaps it) ------
            for i in range(KCH):
                for db in range(D // NB):
                    ps = psp.tile([P, NB], fp32, tag="ps", name="ps")
                    for c in range(DCH):
                        nc.tensor.matmul(
                            ps[:],
                            XT[:, c * S + i * P: c * S + (i + 1) * P],
                            Wv[:, c * D + db * NB: c * D + (db + 1) * NB],
                            start=(c == 0), stop=(c == DCH - 1))
                    psum_copy(
                        V[:, i * D + db * NB: i * D + (db + 1) * NB],
                        ps[:])
            # OT reuses QT's slot (QT dead after the last scores matmul);
            # same shape, so the tag alias is exact.
            OT = pp.tile([P, DCH * SQ], bf16, name="OT", tag="QT")
            #    [p, j*SQ+q] = out_pre[q, j*128+p]

            # Wo reuses KT's slot (KT dead after the last scores matmul);
            # issued after the mask DMAs so they are not delayed.
            Wo = pp.tile([P, DCH * D], bf16, name="Wo", tag="KT")
            d_wo = wload(Wo, wo_d)
            if use_deps:
                add_dep_helper(d_wo.ins, d_prev.ins, sync=False, reason="dma order")

            # ---- AV (2-tile groups) interleaved with the out projection --
            def g_tile(t):
                ot = wkp.tile([P, D], fp32, tag="ot", name="ot", bufs=1)
                for db in range(D // NB):
                    ps = psp.tile([P, NB], fp32, tag="ps", name="ps")
                    for c in range(DCH):
                        nc.tensor.matmul(
                            ps[:],
                            OT[:, c * SQ + t * P: c * SQ + (t + 1) * P],
                            Wo[:, c * D + db * NB: c * D + (db + 1) * NB],
                            start=(c == 0), stop=(c == DCH - 1))
                    psum_copy(ot[:, db * NB:(db + 1) * NB], ps[:])
                nc.sync.dma_start(out_d.ap()[t * P:(t + 1) * P, :], ot[:])

            GB = 2 * P   # AV group width (2 q-tiles)
            for g in range(SQ // GB):
                for j in range(DCH):
                    ps = psav.tile([P, GB], fp32, tag="av", name="av")
                    for i in range(KCH):
                        nc.tensor.matmul(
                            ps[:],
                            V[:, i * D + j * P: i * D + (j + 1) * P],
                            WT[:, i * SQ + g * GB: i * SQ + (g + 1) * GB],
                            start=(i == 0), stop=(i == KCH - 1))
                    psum_copy(
                        OT[:, j * SQ + g * GB: j * SQ + (g + 1) * GB],
                        ps[:])
                for t in range(g * GB // P, (g + 1) * GB // P):
                    g_tile(t)
            wk_ctx.__exit__(None, None, None)
            wv_ctx.__exit__(None, None, None)
            sx_ctx.__exit__(None, None, None)

    nc.compile()
    return nc


def _get_nc():
    if "nc" not in _CACHE:
        _CACHE["nc"] = build()
    return _CACHE["nc"]


def _prep_inputs(x, stride_masks, Wq, Wk, Wv, Wo):
    """Host-side dtype/layout prep (no math beyond the Wo * 1/M fold)."""
    from ml_dtypes import bfloat16

    QTILES = SQ // PART

    wq = np.ascontiguousarray(Wq.astype(bfloat16))
    wk = np.ascontiguousarray(Wk.astype(bfloat16))
    wv = np.ascontiguousarray(Wv.astype(bfloat16))
    wo = np.ascontiguousarray((Wo / np.float32(M)).astype(bfloat16))

    # xT per (batch, half): own query-half columns first (key permutation)
    xT = {}
    for b in range(B):
        xb = np.ascontiguousarray(x[b].T.astype(bfloat16))  # [D, S]
        xT[(b, 0)] = xb
        xT[(b, 1)] = np.ascontiguousarray(
            np.concatenate([xb[:, SQ:], xb[:, :SQ]], axis=1))

    # masks: uint8, per-half q slice, same key permutation, tile layout
    m8 = stride_masks.astype(np.uint8)  # [M, S, S]
    mk = {}
    for h in range(2):
        v = m8[:, h * SQ:(h + 1) * SQ, :]                    # [M, SQ, S]
        if h == 1:
            v = np.concatenate([v[:, :, SQ:], v[:, :, :SQ]], axis=2)
        v = v.transpose(1, 0, 2).reshape(QTILES, PART, M * S)
        mk[h] = np.ascontiguousarray(v)
    return wq, wk, wv, wo, xT, mk


def kernel(x, stride_masks, Wq, bq, Wk, bk, Wv, bv, Wo, bo):
    from concourse import bass_utils

    x = np.ascontiguousarray(np.asarray(x, dtype=np.float32))
    stride_masks = np.ascontiguousarray(np.asarray(stride_masks, dtype=np.int32))
    Wq = np.asarray(Wq, dtype=np.float32)
    Wk = np.asarray(Wk, dtype=np.float32)
    Wv = np.asarray(Wv, dtype=np.float32)
    Wo = np.asarray(Wo, dtype=np.float32)
    bq = np.asarray(bq, dtype=np.float32)
    bk = np.asarray(bk, dtype=np.float32)
    bv = np.asarray(bv, dtype=np.float32)
    bo = np.asarray(bo, dtype=np.float32)

    nc = _get_nc()

    # Biases are spec'd zero-fill; the device kernel omits them. bv/bo fold
    # in exactly on the host (softmax rows sum to 1); bq/bk would need a
    # device path, so assert they are zero.
    assert not (np.any(bq) or np.any(bk)), "nonzero q/k bias unsupported"

    wq, wk, wv, wo, xT, mk = _prep_inputs(x, stride_masks, Wq, Wk, Wv, Wo)

    in_maps = []
    for c in range(N_CORES):
        b, h = c // 2, c % 2
        in_maps.append({
            "xT": xT[(b, h)], "mk": mk[h],
            "Wq": wq, "Wk": wk, "Wv": wv, "Wo": wo,
        })

    res = bass_utils.run_bass_kernel_spmd(nc, in_maps, core_ids=list(range(N_CORES)))
    _CACHE["last_results"] = res

    out = np.empty((B, S, D), dtype=np.float32)
    for c in range(N_CORES):
        b, h = c // 2, c % 2
        out[b, h * SQ:(h + 1) * SQ, :] = res.results[c]["out"]

    if np.any(bv):
        out += (bv @ Wo)[None, None, :]
    if np.any(bo):
        out += bo[None, None, :]
    return out


# revision 14
# speedup vs baseline: 1.0745x; 1.0745x over previous
"""CAMSA multi-mask attention kernel for one TRN2 chip (8 NeuronCores).

Problem: B=4, S=2048, D=1024, M=4 stride masks.
  Q = x@Wq + bq ; K = x@Wk + bk ; V = x@Wv + bv     (biases are zero-fill)
  scores = Q K^T / sqrt(D)                           [B,S,S]
  weights_m = softmax(where(mask_m==0, -1e9, scores))
  out = (mean_m weights_m) @ V @ Wo + bo

Algebra: with P = exp(scores/sqrt(D)) (no row-max needed; scores ~ N(0,1)):
  den_m[q] = sum_k mask_m[q,k] P[q,k];  inv_m = 1/den_m
  Wsum = sum_m inv_m * (mask_m*P);  out = Wsum @ V @ (Wo/M)
(the 1/M mean is folded into Wo on the host).

Sharding: core c = (batch b=c//2, query-half h=c%2): 1024 query rows,
full 2048 keys; K/V projections duplicated within a batch pair (no
collectives).  Host-side prep is pure dtype/layout: bf16 casts of
x/W (the device would DMA-cast anyway), masks int32 -> uint8 in a
per-q-tile layout, and a per-core "own half first" permutation of the
key axis (applied consistently to x columns and mask k) so one SPMD
program serves both halves without a separate xTq input.

Device pipeline per core (all matmuls bf16, contraction on partitions):
  tensor order: Q proj, K proj, V proj, scores t=0..7, AV 2-tile
  groups interleaved with the Wo projection per q-tile.
  per q-tile softmax chain under the matmul stream:
    ACT: P=exp(scores) from PSUM (4 blocks), 2 scaled copies
    DVE: 3x STT(mask*P, accum->den), recip, 2x TS(4x), 3x TT(2x)
    GpSimd: 1x STT(mask*P, accum->den)
    sync-DMA: Wsum -> WT transpose (xbar), outputs
"""

import numpy as np

B, S, D, M = 4, 2048, 1024, 4
SQ = S // 2          # query rows per core
PART = 128
N_CORES = 8

_CACHE = {}


def build(nc_factory=None, S=S, D=D, SQ=SQ, M=M, use_deps=True):
    from concourse import bass, mybir, bacc, tile
    from concourse.tile import add_dep_helper

    fp32 = mybir.dt.float32
    bf16 = mybir.dt.bfloat16
    u8 = mybir.dt.uint8
    AF = mybir.ActivationFunctionType
    ALU = mybir.AluOpType

    P = PART
    DCH = D // P         # d-chunks (8)
    KCH = S // P         # key-row chunks (16)
    QTILES = SQ // P     # q-tiles per core (8)
    NB = 512
    GB = 2 * PART     # AV group width (2 q-tiles)

    if nc_factory is None:
        nc = bacc.Bacc("TRN2", target_bir_lowering=False, debug=False,
                       num_devices=N_CORES)
    else:
        nc = nc_factory()

    xT_d = nc.dram_tensor("xT", [D, S], bf16, kind="ExternalInput")
    mk_d = nc.dram_tensor("mk", [QTILES, P, M * S], u8, kind="ExternalInput")
    wq_d = nc.dram_tensor("Wq", [D, D], bf16, kind="ExternalInput")
    wk_d = nc.dram_tensor("Wk", [D, D], bf16, kind="ExternalInput")
    wv_d = nc.dram_tensor("Wv", [D, D], bf16, kind="ExternalInput")
    wo_d = nc.dram_tensor("Wo", [D, D], bf16, kind="ExternalInput")
    out_d = nc.dram_tensor("out", [SQ, D], fp32, kind="ExternalOutput")

    with tile.TileContext(nc) as tc:
        with tc.tile_pool(name="persist", bufs=1) as pp, \
             tc.tile_pool(name="psum", bufs=6, space="PSUM") as psp, \
             tc.tile_pool(name="psav", bufs=2, space="PSUM") as psav:

            QT = pp.tile([P, DCH * SQ], bf16, tag="QT")  # [p, j*SQ+q] = Q[q, j*128+p]
            KT = pp.tile([P, DCH * S], bf16, tag="KT")  # [p,j*S+k] = K[k,j*128+p]
            V = pp.tile([P, KCH * D], bf16)      # [p, i*D+d]  = V[i*128+p, d]

            def wload(dst, src_d):
                return nc.gpsimd.dma_start(
                    dst[:].rearrange("p (c d) -> p c d", c=DCH),
                    src_d.ap().rearrange("(c p) d -> p c d", p=P))

            # PSUM -> SBUF copy engines, round-robined so no engine stalls
            # the tensor stream (GPSIMD cannot access PSUM).
            cp_engines = [nc.vector, nc.scalar]
            cp_idx = [0]

            def psum_copy(dst_ap, src_ap):
                eng = cp_engines[cp_idx[0] % 2]
                cp_idx[0] += 1
                if eng is nc.scalar:
                    eng.copy(dst_ap, src_ap)
                else:
                    eng.tensor_copy(dst_ap, src_ap)

            def proj(dst, w_sb, src_sb, ncols, src_off=0):
                # dst[p, j*ncols+r] = sum_dx W[dx, j*128+p] * src[dx, src_off+r]
                for j in range(DCH):
                    for qb in range(ncols // NB):
                        ps = psp.tile([P, NB], fp32, tag="ps", name="ps")
                        for c in range(DCH):
                            nc.tensor.matmul(
                                ps[:],
                                w_sb[:, c * D + j * P: c * D + (j + 1) * P],
                                src_sb[:, c * S + src_off + qb * NB:
                                       c * S + src_off + (qb + 1) * NB],
                                start=(c == 0), stop=(c == DCH - 1))
                        psum_copy(
                            dst[:, j * ncols + qb * NB: j * ncols + (qb + 1) * NB],
                            ps[:])

            # ---- load + Q/K projections --------------------------------
            sx_ctx = tc.tile_pool(name="stage_x", bufs=1)
            sx = sx_ctx.__enter__()
            XT = sx.tile([P, DCH * S], bf16, name="XT")
            wqk_ctx = tc.tile_pool(name="stage_wqk", bufs=1)
            swqk = wqk_ctx.__enter__()
            Wq = swqk.tile([P, DCH * D], bf16, name="Wq")
            Wk = swqk.tile([P, DCH * D], bf16, name="Wk")
            d_wq = wload(Wq, wq_d)
            # own-half query columns first (host permutation): split the XT
            # load so the Q projection can start after ~4MB of DMA.
            d_xh = nc.gpsimd.dma_start(
                XT[:].rearrange("p (c r) -> p c r", c=DCH)[:, :, 0:SQ],
                xT_d.ap().rearrange("(c p) r -> p c r", p=P)[:, :, 0:SQ])
            d_xt = nc.gpsimd.dma_start(
                XT[:].rearrange("p (c r) -> p c r", c=DCH)[:, :, SQ:S],
                xT_d.ap().rearrange("(c p) r -> p c r", p=P)[:, :, SQ:S])
            d_wk = wload(Wk, wk_d)
            if use_deps:
                add_dep_helper(d_xh.ins, d_wq.ins, sync=False, reason="dma order")
                add_dep_helper(d_xt.ins, d_xh.ins, sync=False, reason="dma order")
                add_dep_helper(d_wk.ins, d_xt.ins, sync=False, reason="dma order")

            # warm the exp activation table during the projection phase
            with tc.tile_pool(name="warm", bufs=1) as wpool:
                wt = wpool.tile([P, 2], fp32)
                nc.vector.memset(wt[:], 0.0)
                nc.scalar.activation(wt[:], wt[:], AF.Exp)

            proj(QT, Wq, XT, SQ, src_off=0)
            proj(KT, Wk, XT, S, src_off=0)
            wqk_ctx.__exit__(None, None, None)

            # Wv reuses the Wq/Wk space (pool opened after wqk closes); its
            # load waits for the last Wq/Wk reader automatically.
            wv_ctx = tc.tile_pool(name="stage_wv", bufs=1)
            swv = wv_ctx.__enter__()
            Wv = swv.tile([P, DCH * D], bf16, name="Wv")
            d_wv = wload(Wv, wv_d)
            if use_deps:
                add_dep_helper(d_wv.ins, d_wk.ins, sync=False, reason="dma order")
            d_prev = d_wv

            # ---- work pools for the softmax/AV/out phases ----------------
            wk_ctx = tc.tile_pool(name="work", bufs=2)
            wkp = wk_ctx.__enter__()

            # ---- scores -> P -> masked softmax -> WsumT ------------------
            # V projection i-chunks are interleaved into the loop (2 per
            # q-tile) so the tensor engine fills the DVE-paced gaps.
            inv_scale = 1.0 / float(np.sqrt(np.float32(D)))
            wtg_tiles = []
            for t in range(QTILES):
                mt = wkp.tile([P, M * S], bf16, tag="mt", name="mt")
                d_mt = nc.gpsimd.dma_start(mt[:], mk_d.ap()[t])
                if use_deps:
                    add_dep_helper(d_mt.ins, d_prev.ins, sync=False,
                                   reason="mask order")
                    d_prev = d_mt

                Pt = wkp.tile([P, S], bf16, tag="Pt", name="Pt")
                for kb in range(S // NB):
                    ps = psp.tile([P, NB], fp32, tag="ps", name="ps")
                    for c in range(DCH):
                        nc.tensor.matmul(
                            ps[:],
                            QT[:, c * SQ + t * P: c * SQ + (t + 1) * P],
                            KT[:, c * S + kb * NB: c * S + (kb + 1) * NB],
                            start=(c == 0), stop=(c == DCH - 1))
                    nc.scalar.activation(
                        Pt[:, kb * NB:(kb + 1) * NB], ps[:],
                        AF.Exp, scale=inv_scale)

                for i in (2 * t, 2 * t + 1):
                    for db in range(D // NB):
                        ps = psp.tile([P, NB], fp32, tag="ps", name="ps")
                        for c in range(DCH):
                            nc.tensor.matmul(
                                ps[:],
                                XT[:, c * S + i * P: c * S + (i + 1) * P],
                                Wv[:, c * D + db * NB: c * D + (db + 1) * NB],
                                start=(c == 0), stop=(c == DCH - 1))
                        psum_copy(
                            V[:, i * D + db * NB: i * D + (db + 1) * NB],
                            ps[:])

                den = wkp.tile([P, M], fp32, tag="den", name="den")
                # fused product + row-sum per mask; in-place T_m = mask_m*P.
                # (STT only exists on DVE; GPSIMD takes the final add below.)
                for m in range(M):
                    eng = nc.vector
                    eng.scalar_tensor_tensor(
                        out=mt[:, m * S:(m + 1) * S],
                        in0=mt[:, m * S:(m + 1) * S],
                        scalar=1.0, in1=Pt[:],
                        op0=ALU.mult, op1=ALU.mult,
                        accum_out=den[:, m:m + 1])
                inv = wkp.tile([P, M], fp32, tag="inv", name="inv")
                nc.vector.reciprocal(inv[:], den[:])

                # Wsum = sum_m inv_m * T_m as a balanced tree:
                #   A = inv0*T0 (ACT)   Bc = inv1*T1 (DVE TS 4x)
                #   C = inv2*T2 (ACT)   Dc = inv3*T3 (DVE TS 4x)
                #   A += Bc ; C += Dc ; A += C   (DVE TT 2x)
                A = wkp.tile([P, S], bf16, tag="A", name="A", bufs=1)
                Bc = wkp.tile([P, S], bf16, tag="Bc", name="Bc", bufs=1)
                C = wkp.tile([P, S], bf16, tag="C", name="C", bufs=1)
                Dc = wkp.tile([P, S], bf16, tag="Bc", name="Bc", bufs=1)
                nc.scalar.activation(A[:], mt[:, 0:S], AF.Copy,
                                     scale=inv[:, 0:1])
                nc.vector.tensor_scalar(Bc[:], mt[:, S:2 * S],
                                        inv[:, 1:2], None, ALU.mult)
                nc.scalar.activation(C[:], mt[:, 2 * S:3 * S], AF.Copy,
                                     scale=inv[:, 2:3])
                nc.vector.tensor_scalar(Dc[:], mt[:, 3 * S:4 * S],
                                        inv[:, 3:4], None, ALU.mult)
                nc.vector.tensor_tensor(A[:], A[:], Bc[:], op=ALU.add)
                nc.vector.tensor_tensor(C[:], C[:], Dc[:], op=ALU.add)
                nc.gpsimd.tensor_tensor(A[:], A[:], C[:], op=ALU.add)

                # transpose Wsum [128, S] -> WTg columns via xbar DMA
                if t % 2 == 0:
                    wtg_tiles.append(
                        wkp.tile([P, KCH * GB], bf16, tag="WTg", name="WTg",
                                 bufs=3))
                wtg = wtg_tiles[t // 2]
                lt = (t % 2) * P
                nc.sync.dma_start_transpose(
                    wtg[:].rearrange("p (i q) -> p i q", i=KCH)[:, :, lt:lt + P],
                    A[:])

            # OT reuses QT's slot (QT dead after the last scores matmul);
            # same shape, so the tag alias is exact.
            OT = pp.tile([P, DCH * SQ], bf16, name="OT", tag="QT")
            #    [p, j*SQ+q] = out_pre[q, j*128+p]

            # Wo reuses KT's slot (KT dead after the last scores matmul);
            # issued after the mask DMAs so they are not delayed.
            Wo = pp.tile([P, DCH * D], bf16, name="Wo", tag="KT")
            d_wo = wload(Wo, wo_d)
            if use_deps:
                add_dep_helper(d_wo.ins, d_prev.ins, sync=False, reason="dma order")

            # ---- AV (2-tile groups) interleaved with the out projection --
            def g_tile(t):
                ot = wkp.tile([P, D], bf16, tag="ot", name="ot", bufs=1)
                for db in range(D // NB):
                    ps = psp.tile([P, NB], fp32, tag="ps", name="ps")
                    for c in range(DCH):
                        nc.tensor.matmul(
                            ps[:],
                            OT[:, c * SQ + t * P: c * SQ + (t + 1) * P],
                            Wo[:, c * D + db * NB: c * D + (db + 1) * NB],
                            start=(c == 0), stop=(c == DCH - 1))
                    psum_copy(ot[:, db * NB:(db + 1) * NB], ps[:])
                nc.gpsimd.dma_start(out_d.ap()[t * P:(t + 1) * P, :], ot[:])

            for g in range(SQ // GB):
                for j in range(DCH):
                    ps = psav.tile([P, GB], fp32, tag="av", name="av")
                    for i in range(KCH):
                        nc.tensor.matmul(
                            ps[:],
                            V[:, i * D + j * P: i * D + (j + 1) * P],
                            wtg_tiles[g][:, i * GB:(i + 1) * GB],
                            start=(i == 0), stop=(i == KCH - 1))
                    psum_copy(
                        OT[:, j * SQ + g * GB: j * SQ + (g + 1) * GB],
                        ps[:])
                for t in range(g * GB // P, (g + 1) * GB // P):
                    g_tile(t)
            wk_ctx.__exit__(None, None, None)
            wv_ctx.__exit__(None, None, None)
            sx_ctx.__exit__(None, None, None)

    nc.compile()
    return nc


def _get_nc():
    if "nc" not in _CACHE:
        _CACHE["nc"] = build()
    return _CACHE["nc"]


def _prep_inputs(x, stride_masks, Wq, Wk, Wv, Wo):
    """Host-side dtype/layout prep (no math beyond the Wo * 1/M fold)."""
    from ml_dtypes import bfloat16

    QTILES = SQ // PART

    wq = np.ascontiguousarray(Wq.astype(bfloat16))
    wk = np.ascontiguousarray(Wk.astype(bfloat16))
    wv = np.ascontiguousarray(Wv.astype(bfloat16))
    wo = np.ascontiguousarray((Wo / np.float32(M)).astype(bfloat16))

    # xT per (batch, half): own query-half columns first (key permutation)
    xT = {}
    for b in range(B):
        xb = np.ascontiguousarray(x[b].T.astype(bfloat16))  # [D, S]
        xT[(b, 0)] = xb
        xT[(b, 1)] = np.ascontiguousarray(
            np.concatenate([xb[:, SQ:], xb[:, :SQ]], axis=1))

    # masks: uint8, per-half q slice, same key permutation, tile layout
    m8 = stride_masks.astype(np.uint8)  # [M, S, S]
    mk = {}
    for h in range(2):
        v = m8[:, h * SQ:(h + 1) * SQ, :]                    # [M, SQ, S]
        if h == 1:
            v = np.concatenate([v[:, :, SQ:], v[:, :, :SQ]], axis=2)
        v = v.transpose(1, 0, 2).reshape(QTILES, PART, M * S)
        mk[h] = np.ascontiguousarray(v)
    return wq, wk, wv, wo, xT, mk


def kernel(x, stride_masks, Wq, bq, Wk, bk, Wv, bv, Wo, bo):
    from concourse import bass_utils

    x = np.ascontiguousarray(np.asarray(x, dtype=np.float32))
    stride_masks = np.ascontiguousarray(np.asarray(stride_masks, dtype=np.int32))
    Wq = np.asarray(Wq, dtype=np.float32)
    Wk = np.asarray(Wk, dtype=np.float32)
    Wv = np.asarray(Wv, dtype=np.float32)
    Wo = np.asarray(Wo, dtype=np.float32)
    bq = np.asarray(bq, dtype=np.float32)
    bk = np.asarray(bk, dtype=np.float32)
    bv = np.asarray(bv, dtype=np.float32)
    bo = np.asarray(bo, dtype=np.float32)

    nc = _get_nc()

    # Biases are spec'd zero-fill; the device kernel omits them. bv/bo fold
    # in exactly on the host (softmax rows sum to 1); bq/bk would need a
    # device path, so assert they are zero.
    assert not (np.any(bq) or np.any(bk)), "nonzero q/k bias unsupported"

    wq, wk, wv, wo, xT, mk = _prep_inputs(x, stride_masks, Wq, Wk, Wv, Wo)

    in_maps = []
    for c in range(N_CORES):
        b, h = c // 2, c % 2
        in_maps.append({
            "xT": xT[(b, h)], "mk": mk[h],
            "Wq": wq, "Wk": wk, "Wv": wv, "Wo": wo,
        })

    res = bass_utils.run_bass_kernel_spmd(nc, in_maps, core_ids=list(range(N_CORES)))
    _CACHE["last_results"] = res

    out = np.empty((B, S, D), dtype=np.float32)
    for c in range(N_CORES):
        b, h = c // 2, c % 2
        out[b, h * SQ:(h + 1) * SQ, :] = res.results[c]["out"]

    if np.any(bv):
        out += (bv @ Wo)[None, None, :]
    if np.any(bo):
        out += bo[None, None, :]
    return out
